# revision 1
# baseline (speedup 1.0000x reference)
"""Trainium2 Bass kernel for nn_DeepLatent loss (chamfer + L2 of a per-point MLP).

Strategy (8 cores, data-parallel over batch B=32 -> 4 samples/core):
  Per core, per sample s (channel-major layout: activations stored [C, Npoints]):
    h1 = relu(W1o.T @ obs^T + latbias)        latbias = W1lat.T @ latent + b1 (tiny matmul)
    h2 = relu(W2.T @ h1 + b2)
    h3 = relu(W3.T @ h2 + b3)
    delta = W4.T @ h3                         est = obs + delta + b4
  Chamfer via augmented grams (K=6 matmuls: 3 coord rows + 3 aux rows):
    G [n,m]  = gt_n . est_m - |est_m|^2/2     (aux lhsT rows = -0.5, aux rhs rows = est^2)
    G'[m,n]  = est_m . gt_n - |gt_n|^2/2
    min_m d2[n,m] = |gt_n|^2 - 2 max_m G[n,m]   (max via fused DVE tensor_tensor_reduce)
  Per-core partial sums (max-sums, sq-sums, cross-sum) are combined on the host.

All matmuls use float32r (fp22 truncation, 1 cycle/col at free-dim>=256).
"""

import ml_dtypes
import numpy as np
from contextlib import ExitStack

import concourse.bass as bass
import concourse.bacc as bacc
import concourse.mybir as mybir
import concourse.tile as tile
from concourse.bass_utils import run_bass_kernel_spmd

F32 = mybir.dt.float32
F32R = mybir.dt.float32r
BF16 = mybir.dt.bfloat16
AX = mybir.AxisListType
OP = mybir.AluOpType
ACTF = mybir.ActivationFunctionType

B, N, L = 32, 1024, 256
NCORES = 8
BS = B // NCORES  # samples per core
NT = N // 128     # n-tiles per sample
NEG = -3.0e38

# test.py hooks
TRACE = False
LAST = None


def _r(ap):
    return ap.bitcast(F32R)


def build_program(do_mlp=True, do_gram=True):
    nc = bacc.Bacc()

    # host-pretransposed layouts: every DMA below is inner-contiguous
    obs_d = nc.dram_tensor("obs_t", [3, BS, N], BF16, kind="ExternalInput")[:]
    gt_d = nc.dram_tensor("gt_t", [3, BS, N], BF16, kind="ExternalInput")[:]
    lat_d = nc.dram_tensor("lat_t", [L, BS], F32, kind="ExternalInput")[:]
    W1od = nc.dram_tensor("w1o", [3, 512], BF16, kind="ExternalInput")[:]
    eye3d = nc.dram_tensor("eye3", [3, 3], BF16, kind="ExternalInput")[:]
    W1ld = nc.dram_tensor("w1l", [128, 2, 512], F32, kind="ExternalInput")[:]
    b1d = nc.dram_tensor("b1r", [1, 512], F32, kind="ExternalInput")[:]
    W2d = nc.dram_tensor("w2p", [128, 4, 512], BF16, kind="ExternalInput")[:]
    b2d = nc.dram_tensor("b2p", [128, 4], F32, kind="ExternalInput")[:]
    W3d = nc.dram_tensor("w3p", [128, 4, 256], BF16, kind="ExternalInput")[:]
    b3d = nc.dram_tensor("b3p", [128, 2], F32, kind="ExternalInput")[:]
    W4d = nc.dram_tensor("w4p", [128, 2, 3], BF16, kind="ExternalInput")[:]
    b4d = nc.dram_tensor("b4p", [3, 1], F32, kind="ExternalInput")[:]
    out_d = nc.dram_tensor("partials", [1, 8], F32, kind="ExternalOutput")[:]

    with tile.TileContext(nc) as tc, ExitStack() as ctx:
        singles = ctx.enter_context(tc.tile_pool(name="singles", bufs=1))

        def fixed(shape, name, dtype=F32):
            return singles.tile(shape, dtype, tag=name, name=name)

        # ---------- fixed tiles ----------
        w1o = fixed([3, 512], "w1o", BF16)
        eye3 = fixed([3, 3], "eye3", BF16)
        w1l = fixed([128, 2, 512], "w1l")
        b1r = fixed([1, 512], "b1r")
        w2t = fixed([128, 4, 512], "w2t", BF16)
        w3t = fixed([128, 4, 256], "w3t", BF16)
        w4t = fixed([128, 2, 3], "w4t", BF16)
        b2t = fixed([128, 4], "b2t")
        b3t = fixed([128, 2], "b3t")
        b4p = fixed([3, 1], "b4p")
        latT = fixed([128, 2, BS], "latT")
        ones_r = fixed([1, BS], "ones_r")
        ones_c = fixed([128, 1], "ones_c")
        latb = fixed([128, 4, BS], "latb")
        Pg = fixed([3, BS, N], "Pg", BF16)
        Pe = fixed([3, BS, N], "Pe", BF16)
        Pg2s = fixed([3, N], "Pg2s", BF16)
        Pe2s = fixed([3, N], "Pe2s", BF16)
        SGS = fixed([3, BS], "SGS")
        neghalf = fixed([3, N], "neghalf", BF16)
        M1 = fixed([128, BS * NT], "M1")
        M2 = fixed([128, BS * NT], "M2")
        Ft = fixed([128, 8], "Ft")
        SES = fixed([3, BS], "SES")
        ttr_dump = fixed([128, 512], "ttr_dump")
        outs = fixed([1, 8], "outs")
        A_ = [fixed([128, N], f"Areg{i}", BF16) for i in range(2)]
        B_ = [fixed([128, N], f"Breg{i}", BF16) for i in range(2)]
        C_ = [fixed([128, N], f"Creg{i}", BF16) for i in range(2)]
        D_ = [fixed([128, N], f"Dreg{i}", BF16) for i in range(2)]

        h1p = ctx.enter_context(tc.tile_pool(name="h1", bufs=2))
        h2p = ctx.enter_context(tc.tile_pool(name="h2", bufs=2))
        h3p = ctx.enter_context(tc.tile_pool(name="h3", bufs=2))
        otp = ctx.enter_context(tc.tile_pool(name="obsT", bufs=2))
        psA = ctx.enter_context(tc.tile_pool(name="psA", bufs=2, space="PSUM"))
        psG = ctx.enter_context(tc.tile_pool(name="psG", bufs=2, space="PSUM"))

        # ---------- startup ----------
        nc.sync.dma_start(out=w1o, in_=W1od)
        nc.sync.dma_start(out=eye3, in_=eye3d)
        for k in range(2):
            nc.sync.dma_start(out=latT[:, k, :], in_=lat_d[128 * k:128 * (k + 1), :])
        nc.sync.dma_start(out=b1r, in_=b1d)
        nc.sync.dma_start(out=w1l, in_=W1ld)
        nc.sync.dma_start(out=b2t, in_=b2d)
        nc.sync.dma_start(out=b3t, in_=b3d)
        nc.sync.dma_start(out=b4p, in_=b4d)
        nc.sync.dma_start(out=Pg, in_=gt_d)
        nc.sync.dma_start(out=w2t, in_=W2d)
        nc.sync.dma_start(out=w3t, in_=W3d)
        nc.sync.dma_start(out=w4t, in_=W4d)
        nc.vector.memset(ones_r, 1.0)
        nc.vector.memset(ones_c, 1.0)
        nc.vector.memset(Ft, 0.0)
        # aux lhsT rows {3-5, 35-37} of A/B must be -0.5: memset an fp32
        # staging row-band, then DMA it in (f32r-tagged) since compute engines
        # cannot emit float32r directly.
        nc.vector.memset(neghalf, -0.5)
        for t_ in A_ + B_:
            for g in range(2):
                nc.gpsimd.dma_start(out=t_[32 * g + 3:32 * g + 6, :],
                                    in_=neghalf[:, :])

        # latent bias vectors: latb[cout, c-tile, s] = (latent @ W1[3:] + b1)^T
        for c in range(4):
            lps = psG.tile([128, 1024], F32, tag="g", name=f"latps{c}")
            for k in range(2):
                nc.tensor.matmul(lps[:, 0:BS], w1l[:, k, 128 * c:128 * (c + 1)],
                                 latT[:, k, :], start=(k == 0), stop=False)
            nc.tensor.matmul(lps[:, 0:BS], b1r[:, 128 * c:128 * (c + 1)],
                             ones_r[:, :], start=False, stop=True)
            nc.vector.tensor_copy(latb[:, c, :], lps[:, 0:BS])

        # ---------- per-sample gram rounds (generator; interleaved with next MLP) ----------
        def gram_rounds(s):
            Ar, Br, Cr, Dr = A_[s % 2], B_[s % 2], C_[s % 2], D_[s % 2]
            for lhs_reg, rhs_reg, Mt in ((Ar, Cr, M1), (Br, Dr, M2)):
                for r in range(4):
                    gtiles = []
                    for g in range(2):
                        t = 2 * r + g
                        gp = psG.tile([128, 1024], F32, tag="g", name=f"gp{s}_{r}_{g}")
                        for j in range(2):
                            nc.tensor.matmul(
                                gp[:, 512 * j:512 * (j + 1)],
                                lhs_reg[32 * g:32 * g + 6, 128 * t:128 * (t + 1)],
                                rhs_reg[32 * g:32 * g + 6, 512 * j:512 * (j + 1)],
                                start=True, stop=True)
                        gtiles.append((t, gp))
                    for t, gp in gtiles:
                        nc.vector.tensor_reduce(
                            out=Mt[:, NT * s + t:NT * s + t + 1], in_=gp[:, :],
                            axis=AX.X, op=OP.max)
                    yield

        def advance(it):
            if it is not None:
                next(it, None)

        # ---------- per-sample MLP ----------
        def mlp(s, hooks):
            obsT = otp.tile([3, N], BF16, tag="obsT", name=f"obsT{s}")
            nc.gpsimd.dma_start(out=obsT, in_=obs_d[:, s, :])
            nc.scalar.activation(Pg2s[:, :], Pg[:, s, :], ACTF.Square,
                                 accum_out=SGS[:, s:s + 1])
            Ar, Dr = A_[s % 2], D_[s % 2]
            for g in range(2):
                nc.gpsimd.dma_start(out=Ar[32 * g:32 * g + 3, :], in_=Pg[:, s, :])
                nc.gpsimd.dma_start(out=Dr[32 * g:32 * g + 3, :], in_=Pg[:, s, :])
                nc.gpsimd.dma_start(out=Dr[32 * g + 3:32 * g + 6, :], in_=Pg2s[:, :])

            if not do_mlp:
                # est := gt (copies exercise the same f32r-output DVE/ACT path)
                nc.vector.scalar_tensor_tensor(out=Pe[:, s, :], in0=Pg[:, s, :],
                                               scalar=0.0, in1=Pg[:, s, :],
                                               op0=OP.add, op1=OP.bypass)
                nc.scalar.activation(Pe2s[:, :], Pe[:, s, :], ACTF.Square,
                                     accum_out=SES[:, s:s + 1])
                Br0, Cr0 = B_[s % 2], C_[s % 2]
                for g in range(2):
                    nc.gpsimd.dma_start(out=Br0[32 * g:32 * g + 3, :], in_=Pe[:, s, :])
                    nc.gpsimd.dma_start(out=Cr0[32 * g:32 * g + 3, :], in_=Pe[:, s, :])
                    nc.gpsimd.dma_start(out=Cr0[32 * g + 3:32 * g + 6, :], in_=Pe2s[:, :])
                for _ in range(9):
                    advance(hooks)
                return
            h1t = h1p.tile([128, 4, N], BF16, tag="h1", name=f"h1_{s}")
            for c in range(4):
                ps = psA.tile([128, N], F32, tag="a", name=f"l1ps{s}_{c}")
                for j in range(2):
                    nc.tensor.matmul(ps[:, 512 * j:512 * (j + 1)],
                                     w1o[:, 128 * c:128 * (c + 1)],
                                     obsT[:, 512 * j:512 * (j + 1)],
                                     start=True, stop=True)
                nc.scalar.activation(h1t[:, c, :], ps[:, :], ACTF.Relu,
                                     bias=latb[:, c, s:s + 1])
                advance(hooks)

            h2t = h2p.tile([128, 4, N], BF16, tag="h2", name=f"h2_{s}")
            for c in range(4):
                ps = psA.tile([128, N], F32, tag="a", name=f"l2ps{s}_{c}")
                for j in range(2):
                    for k in range(4):
                        nc.tensor.matmul(ps[:, 512 * j:512 * (j + 1)],
                                         w2t[:, k, 128 * c:128 * (c + 1)],
                                         h1t[:, k, 512 * j:512 * (j + 1)],
                                         start=(k == 0), stop=(k == 3))
                nc.scalar.activation(h2t[:, c, :], ps[:, :], ACTF.Relu,
                                     bias=b2t[:, c:c + 1])
                advance(hooks)

            h3t = h3p.tile([128, 2, N], BF16, tag="h3", name=f"h3_{s}")
            for c in range(2):
                ps = psA.tile([128, N], F32, tag="a", name=f"l3ps{s}_{c}")
                for j in range(2):
                    for k in range(4):
                        nc.tensor.matmul(ps[:, 512 * j:512 * (j + 1)],
                                         w3t[:, k, 128 * c:128 * (c + 1)],
                                         h2t[:, k, 512 * j:512 * (j + 1)],
                                         start=(k == 0), stop=(k == 3))
                nc.scalar.activation(h3t[:, c, :], ps[:, :], ACTF.Relu,
                                     bias=b3t[:, c:c + 1])
                advance(hooks)

            ps4 = psG.tile([128, 1024], F32, tag="g", name=f"l4ps{s}")
            for j in range(2):
                for k in range(2):
                    nc.tensor.matmul(ps4[0:3, 512 * j:512 * (j + 1)],
                                     w4t[:, k, :],
                                     h3t[:, k, 512 * j:512 * (j + 1)],
                                     start=(k == 0), stop=False)
                # obs folded into the same accumulation via identity rows
                nc.tensor.matmul(ps4[0:3, 512 * j:512 * (j + 1)],
                                 eye3[:, :],
                                 obsT[:, 512 * j:512 * (j + 1)],
                                 start=False, stop=True)
            nc.scalar.activation(Pe[:, s, :], ps4[0:3, :], ACTF.Identity,
                                 bias=b4p[:, 0:1])
            nc.scalar.activation(Pe2s[:, :], Pe[:, s, :], ACTF.Square,
                                 accum_out=SES[:, s:s + 1])
            Br, Cr = B_[s % 2], C_[s % 2]
            for g in range(2):
                nc.gpsimd.dma_start(out=Br[32 * g:32 * g + 3, :], in_=Pe[:, s, :])
                nc.gpsimd.dma_start(out=Cr[32 * g:32 * g + 3, :], in_=Pe[:, s, :])
                nc.gpsimd.dma_start(out=Cr[32 * g + 3:32 * g + 6, :], in_=Pe2s[:, :])
            advance(hooks)

        pending = None
        for s in range(BS):
            mlp(s, pending)
            if pending is not None:
                for _ in pending:
                    pass
            if do_gram:
                pending = gram_rounds(s)
        if pending is not None:
            for _ in pending:
                pass

        # ---------- finale ----------
        # cross term sum(gt*est) per coordinate -> Ft col 4 (in-place dump into Pe)
        nc.vector.scalar_tensor_tensor(out=Pe[:, :, :], in0=Pg[:, :, :],
                                       scalar=0.0, in1=Pe[:, :, :],
                                       op0=OP.add, op1=OP.mult,
                                       accum_out=Ft[0:3, 4:5])
        nc.vector.tensor_reduce(out=Ft[0:3, 2:3], in_=SGS[:, :], axis=AX.X, op=OP.add)
        nc.vector.tensor_reduce(out=Ft[0:3, 3:4], in_=SES[:, :], axis=AX.X, op=OP.add)
        nc.vector.tensor_reduce(out=Ft[:, 0:1], in_=M1[:, :], axis=AX.X, op=OP.add)
        nc.vector.tensor_reduce(out=Ft[:, 1:2], in_=M2[:, :], axis=AX.X, op=OP.add)

        fps = psG.tile([128, 1024], F32, tag="g", name="fps")
        nc.tensor.matmul(fps[0:1, 0:8], ones_c[:, :], Ft[:, :],
                         start=True, stop=True)
        nc.scalar.activation(outs[:, :], fps[0:1, 0:8], ACTF.Copy)
        nc.sync.dma_start(out=out_d, in_=outs)

    nc.compile()
    return nc


_program_cache = []


def kernel(**inputs):
    global LAST
    if not _program_cache:
        _program_cache.append(build_program())
    nc = _program_cache[0]

    def f32(x):
        return np.ascontiguousarray(np.asarray(x, dtype=np.float32))

    W1 = np.asarray(inputs["W1"], np.float32)
    W2 = np.asarray(inputs["W2"], np.float32)
    W3 = np.asarray(inputs["W3"], np.float32)
    W4 = np.asarray(inputs["W4"], np.float32)
    shared = {
        "w1o": np.ascontiguousarray(W1[0:3, :].astype(ml_dtypes.bfloat16)),
        "eye3": np.eye(3, dtype=ml_dtypes.bfloat16),
        "w1l": f32(W1[3:259, :].reshape(2, 128, 512).transpose(1, 0, 2)),
        "b1r": f32(np.asarray(inputs["b1"], np.float32).reshape(1, 512)),
        "w2p": np.ascontiguousarray(W2.reshape(4, 128, 512).transpose(1, 0, 2).astype(ml_dtypes.bfloat16)),
        "b2p": f32(np.asarray(inputs["b2"], np.float32).reshape(4, 128).T),
        "w3p": np.ascontiguousarray(W3.reshape(4, 128, 256).transpose(1, 0, 2).astype(ml_dtypes.bfloat16)),
        "b3p": f32(np.asarray(inputs["b3"], np.float32).reshape(2, 128).T),
        "w4p": np.ascontiguousarray(W4.reshape(2, 128, 3).transpose(1, 0, 2).astype(ml_dtypes.bfloat16)),
        "b4p": f32(np.asarray(inputs["b4"], np.float32).reshape(3, 1)),
    }
    in_maps = []
    for c in range(NCORES):
        sl = slice(c * BS, (c + 1) * BS)
        m = dict(shared)
        m["obs_t"] = np.ascontiguousarray(np.asarray(inputs["obs"][sl], np.float32).transpose(2, 0, 1).astype(ml_dtypes.bfloat16))
        m["gt_t"] = np.ascontiguousarray(np.asarray(inputs["obs_gt"][sl], np.float32).transpose(2, 0, 1).astype(ml_dtypes.bfloat16))
        m["lat_t"] = f32(np.asarray(inputs["latent"][sl], np.float32).T)
        in_maps.append(m)

    res = run_bass_kernel_spmd(nc, in_maps, core_ids=list(range(NCORES)),
                               trace=TRACE)
    LAST = res

    parts = np.stack([r["partials"][0] for r in res.results]).astype(np.float64)
    s_max1 = parts[:, 0].sum()
    s_max2 = parts[:, 1].sum()
    s_gt2 = parts[:, 2].sum()
    s_est2 = parts[:, 3].sum()
    s_cross = parts[:, 4].sum()
    chm = (s_gt2 - 2.0 * s_max1) / (B * N) + (s_est2 - 2.0 * s_max2) / (B * N)
    l2 = (s_gt2 - 2.0 * s_cross + s_est2) / (B * N * 3)
    loss = 0.2 * chm + 0.8 * l2
    return np.asarray(loss, dtype=np.float32)



# revision 9
# speedup vs baseline: 1.3047x; 1.3047x over previous
"""Trainium2 Bass kernel for nn_DeepLatent loss (chamfer + L2 of a per-point MLP).

Strategy (8 cores, data-parallel over batch B=32 -> 4 samples/core):
  Per core, per sample s (channel-major layout: activations stored [C, Npoints]):
    h1 = relu(W1o.T @ obs^T + latbias)        latbias = W1lat.T @ latent + b1 (tiny matmul)
    h2 = relu(W2.T @ h1 + b2)
    h3 = relu(W3.T @ h2 + b3)
    delta = W4.T @ h3                         est = obs + delta + b4
  Chamfer via augmented grams (K=6 matmuls: 3 coord rows + 3 aux rows):
    G [n,m]  = gt_n . est_m - |est_m|^2/2     (aux lhsT rows = -0.5, aux rhs rows = est^2)
    G'[m,n]  = est_m . gt_n - |gt_n|^2/2
    min_m d2[n,m] = |gt_n|^2 - 2 max_m G[n,m]   (max via fused DVE tensor_tensor_reduce)
  Per-core partial sums (max-sums, sq-sums, cross-sum) are combined on the host.

All matmuls use float32r (fp22 truncation, 1 cycle/col at free-dim>=256).
"""

import ml_dtypes
import numpy as np
from contextlib import ExitStack

import concourse.bass as bass
import concourse.bacc as bacc
import concourse.mybir as mybir
import concourse.tile as tile
from concourse.bass_utils import run_bass_kernel_spmd

F32 = mybir.dt.float32
F32R = mybir.dt.float32r
BF16 = mybir.dt.bfloat16
F8 = mybir.dt.float8e4
DR = mybir.MatmulPerfMode.DoubleRow
AX = mybir.AxisListType
OP = mybir.AluOpType
ACTF = mybir.ActivationFunctionType

B, N, L = 32, 1024, 256
NCORES = 8
BS = B // NCORES  # samples per core
NT = N // 128     # n-tiles per sample
NEG = -3.0e38

# test.py hooks
TRACE = False
LAST = None


def _r(ap):
    return ap.bitcast(F32R)


def build_program(do_mlp=True, do_gram=True):
    nc = bacc.Bacc()

    # host-pretransposed layouts: every DMA below is inner-contiguous
    obs_d = nc.dram_tensor("obs_t", [3, BS, N], BF16, kind="ExternalInput")[:]
    obs8_d = nc.dram_tensor("obs8", [2, 2, BS, N], F8, kind="ExternalInput")[:]
    gt_d = nc.dram_tensor("gt_t", [3, BS, N], BF16, kind="ExternalInput")[:]
    lat_d = nc.dram_tensor("lat_t", [L, BS], BF16, kind="ExternalInput")[:]
    W1ad = nc.dram_tensor("w1a", [2, 2, 512], F8, kind="ExternalInput")[:]
    eye3d = nc.dram_tensor("eye3", [3, 3], BF16, kind="ExternalInput")[:]
    W1ld = nc.dram_tensor("w1l", [128, 2, 512], BF16, kind="ExternalInput")[:]
    b1d = nc.dram_tensor("b1r", [1, 512], F32, kind="ExternalInput")[:]
    W2d = nc.dram_tensor("w2p", [128, 4, 512], F8, kind="ExternalInput")[:]
    b2d = nc.dram_tensor("b2p", [128, 4], F32, kind="ExternalInput")[:]
    W3d = nc.dram_tensor("w3p", [128, 4, 256], F8, kind="ExternalInput")[:]
    b3d = nc.dram_tensor("b3p", [128, 2], F32, kind="ExternalInput")[:]
    W4d = nc.dram_tensor("w4p", [128, 2, 16], F8, kind="ExternalInput")[:]
    b4d = nc.dram_tensor("b4p", [3, 1], F32, kind="ExternalInput")[:]
    out_d = nc.dram_tensor("partials", [1, 8], F32, kind="ExternalOutput")[:]

    with tile.TileContext(nc) as tc, ExitStack() as ctx:
        singles = ctx.enter_context(tc.tile_pool(name="singles", bufs=1))

        def fixed(shape, name, dtype=F32):
            return singles.tile(shape, dtype, tag=name, name=name)

        # ---------- fixed tiles ----------
        w1a = fixed([2, 2, 512], "w1a", F8)
        obs8 = fixed([2, 2, BS, N], "obs8", F8)
        eye3 = fixed([3, 3], "eye3", BF16)
        w1l = fixed([128, 2, 512], "w1l", BF16)
        b1r = fixed([1, 512], "b1r")
        w2t = fixed([128, 4, 512], "w2t", F8)
        w3t = fixed([128, 4, 256], "w3t", F8)
        w4t = fixed([128, 2, 16], "w4t", F8)
        b2t = fixed([128, 4], "b2t")
        b3t = fixed([128, 2], "b3t")
        b4p = fixed([3, 1], "b4p")
        latT = fixed([128, 2, BS], "latT", BF16)
        ones_r = fixed([1, BS], "ones_r")
        ones_c = fixed([128, 1], "ones_c")
        latb = fixed([128, 4, BS], "latb")
        Pg = fixed([3, BS, N], "Pg", BF16)
        Pe = fixed([3, BS, N], "Pe", BF16)
        Pg2s = fixed([3, N], "Pg2s", BF16)
        Pe2s = fixed([3, N], "Pe2s", BF16)
        SGS = fixed([3, BS], "SGS")
        neghalf = fixed([3, N], "neghalf", BF16)
        M1 = fixed([128, BS * NT], "M1")
        M2 = fixed([128, BS * NT], "M2")
        Ft = fixed([128, 8], "Ft")
        SES = fixed([3, BS], "SES")
        ttr_dump = fixed([128, 512], "ttr_dump")
        outs = fixed([1, 8], "outs")
        A_ = [fixed([128, N], f"Areg{i}", BF16) for i in range(2)]
        B_ = [fixed([128, N], f"Breg{i}", BF16) for i in range(2)]
        C_ = [fixed([128, N], f"Creg{i}", BF16) for i in range(2)]
        D_ = [fixed([128, N], f"Dreg{i}", BF16) for i in range(2)]

        h1p = ctx.enter_context(tc.tile_pool(name="h1", bufs=2))
        h2p = ctx.enter_context(tc.tile_pool(name="h2", bufs=2))
        h3p = ctx.enter_context(tc.tile_pool(name="h3", bufs=2))
        otp = ctx.enter_context(tc.tile_pool(name="obsT", bufs=2))
        psA = ctx.enter_context(tc.tile_pool(name="psA", bufs=2, space="PSUM"))
        psG = ctx.enter_context(tc.tile_pool(name="psG", bufs=2, space="PSUM"))

        # ---------- startup ----------
        nc.sync.dma_start(out=w1a, in_=W1ad)
        nc.sync.dma_start(out=obs8, in_=obs8_d)
        nc.sync.dma_start(out=eye3, in_=eye3d)
        for k in range(2):
            nc.sync.dma_start(out=latT[:, k, :], in_=lat_d[128 * k:128 * (k + 1), :])
        nc.sync.dma_start(out=b1r, in_=b1d)
        nc.sync.dma_start(out=w1l, in_=W1ld)
        nc.sync.dma_start(out=b2t, in_=b2d)
        nc.sync.dma_start(out=b3t, in_=b3d)
        nc.sync.dma_start(out=b4p, in_=b4d)
        nc.sync.dma_start(out=Pg, in_=gt_d)
        nc.sync.dma_start(out=w2t, in_=W2d)
        nc.sync.dma_start(out=w3t, in_=W3d)
        nc.sync.dma_start(out=w4t, in_=W4d)
        nc.vector.memset(ones_r, 1.0)
        nc.vector.memset(ones_c, 1.0)
        nc.vector.memset(Ft, 0.0)
        # aux lhsT rows {3-5, 35-37} of A/B must be -0.5: memset an fp32
        # staging row-band, then DMA it in (f32r-tagged) since compute engines
        # cannot emit float32r directly.
        nc.vector.memset(neghalf, -0.5)
        for t_ in A_ + B_:
            for g in range(2):
                nc.gpsimd.dma_start(out=t_[32 * g + 3:32 * g + 6, :],
                                    in_=neghalf[:, :])

        # latent bias vectors: latb[cout, c-tile, s] = (latent @ W1[3:] + b1)^T
        for c in range(4):
            lps = psG.tile([128, 1024], F32, tag="g", name=f"latps{c}")
            for k in range(2):
                nc.tensor.matmul(lps[:, 0:BS], w1l[:, k, 128 * c:128 * (c + 1)],
                                 latT[:, k, :], start=(k == 0), stop=False)
            nc.tensor.matmul(lps[:, 0:BS], b1r[:, 128 * c:128 * (c + 1)],
                             ones_r[:, :], start=False, stop=True)
            nc.vector.tensor_copy(latb[:, c, :], lps[:, 0:BS])

        # ---------- per-sample gram rounds (generator; interleaved with next MLP) ----------
        def gram_rounds(s):
            Ar, Br, Cr, Dr = A_[s % 2], B_[s % 2], C_[s % 2], D_[s % 2]
            for lhs_reg, rhs_reg, Mt in ((Ar, Cr, M1), (Br, Dr, M2)):
                for r in range(4):
                    gtiles = []
                    for g in range(2):
                        t = 2 * r + g
                        gp = psG.tile([128, 1024], F32, tag="g", name=f"gp{s}_{r}_{g}")
                        for j in range(2):
                            nc.tensor.matmul(
                                gp[:, 512 * j:512 * (j + 1)],
                                lhs_reg[32 * g:32 * g + 6, 128 * t:128 * (t + 1)],
                                rhs_reg[32 * g:32 * g + 6, 512 * j:512 * (j + 1)],
                                start=True, stop=True)
                        gtiles.append((t, gp))
                    for t, gp in gtiles:
                        nc.vector.tensor_reduce(
                            out=Mt[:, NT * s + t:NT * s + t + 1], in_=gp[:, :],
                            axis=AX.X, op=OP.max)
                    yield

        def advance(it):
            if it is not None:
                next(it, None)

        # ---------- per-sample MLP ----------
        def mlp(s, hooks):
            obsT = otp.tile([3, N], BF16, tag="obsT", name=f"obsT{s}")
            nc.gpsimd.dma_start(out=obsT, in_=obs_d[:, s, :])
            nc.scalar.activation(Pg2s[:, :], Pg[:, s, :], ACTF.Square,
                                 accum_out=SGS[:, s:s + 1])
            Ar, Dr = A_[s % 2], D_[s % 2]
            for g in range(2):
                nc.gpsimd.dma_start(out=Ar[32 * g:32 * g + 3, :], in_=Pg[:, s, :])
                nc.gpsimd.dma_start(out=Dr[32 * g:32 * g + 3, :], in_=Pg[:, s, :])
                nc.gpsimd.dma_start(out=Dr[32 * g + 3:32 * g + 6, :], in_=Pg2s[:, :])

            if not do_mlp:
                # est := gt (copies exercise the same f32r-output DVE/ACT path)
                nc.vector.scalar_tensor_tensor(out=Pe[:, s, :], in0=Pg[:, s, :],
                                               scalar=0.0, in1=Pg[:, s, :],
                                               op0=OP.add, op1=OP.bypass)
                nc.scalar.activation(Pe2s[:, :], Pe[:, s, :], ACTF.Square,
                                     accum_out=SES[:, s:s + 1])
                Br0, Cr0 = B_[s % 2], C_[s % 2]
                for g in range(2):
                    nc.gpsimd.dma_start(out=Br0[32 * g:32 * g + 3, :], in_=Pe[:, s, :])
                    nc.gpsimd.dma_start(out=Cr0[32 * g:32 * g + 3, :], in_=Pe[:, s, :])
                    nc.gpsimd.dma_start(out=Cr0[32 * g + 3:32 * g + 6, :], in_=Pe2s[:, :])
                for _ in range(9):
                    advance(hooks)
                return
            h1t = h1p.tile([128, 4, N], F8, tag="h1", name=f"h1_{s}")
            for c in range(4):
                ps = psA.tile([128, N], F32, tag="a", name=f"l1ps{s}_{c}")
                for j in range(2):
                    nc.tensor.matmul(ps[:, 512 * j:512 * (j + 1)],
                                     w1a[:, :, 128 * c:128 * (c + 1)],
                                     obs8[:, :, s, 512 * j:512 * (j + 1)],
                                     start=True, stop=True, perf_mode=DR)
                nc.scalar.activation(h1t[:, c, :], ps[:, :], ACTF.Relu,
                                     bias=latb[:, c, s:s + 1])
                advance(hooks)

            h2t = h2p.tile([128, 4, N], F8, tag="h2", name=f"h2_{s}")
            for c in range(4):
                ps = psA.tile([128, N], F32, tag="a", name=f"l2ps{s}_{c}")
                for j in range(2):
                    for k in range(2):
                        nc.tensor.matmul(ps[:, 512 * j:512 * (j + 1)],
                                         w2t[:, 2 * k:2 * k + 2, 128 * c:128 * (c + 1)],
                                         h1t[:, 2 * k:2 * k + 2, 512 * j:512 * (j + 1)],
                                         start=(k == 0), stop=(k == 1), perf_mode=DR)
                nc.scalar.activation(h2t[:, c, :], ps[:, :], ACTF.Relu,
                                     bias=b2t[:, c:c + 1])
                advance(hooks)

            h3t = h3p.tile([128, 2, N], F8, tag="h3", name=f"h3_{s}")
            for c in range(2):
                ps = psA.tile([128, N], F32, tag="a", name=f"l3ps{s}_{c}")
                for j in range(2):
                    for k in range(2):
                        nc.tensor.matmul(ps[:, 512 * j:512 * (j + 1)],
                                         w3t[:, 2 * k:2 * k + 2, 128 * c:128 * (c + 1)],
                                         h2t[:, 2 * k:2 * k + 2, 512 * j:512 * (j + 1)],
                                         start=(k == 0), stop=(k == 1), perf_mode=DR)
                nc.scalar.activation(h3t[:, c, :], ps[:, :], ACTF.Relu,
                                     bias=b3t[:, c:c + 1])
                advance(hooks)

            ps4 = psG.tile([128, 1024], F32, tag="g", name=f"l4ps{s}")
            for j in range(2):
                nc.tensor.matmul(ps4[0:3, 512 * j:512 * (j + 1)],
                                 w4t[:, :, 0:3],
                                 h3t[:, :, 512 * j:512 * (j + 1)],
                                 start=True, stop=False, perf_mode=DR,
                                 skip_group_check=True)
                # obs folded into the same accumulation via identity rows
                nc.tensor.matmul(ps4[0:3, 512 * j:512 * (j + 1)],
                                 eye3[:, :],
                                 obsT[:, 512 * j:512 * (j + 1)],
                                 start=False, stop=True,
                                 skip_group_check=True)
            nc.scalar.activation(Pe[:, s, :], ps4[0:3, :], ACTF.Identity,
                                 bias=b4p[:, 0:1])
            nc.scalar.activation(Pe2s[:, :], Pe[:, s, :], ACTF.Square,
                                 accum_out=SES[:, s:s + 1])
            Br, Cr = B_[s % 2], C_[s % 2]
            for g in range(2):
                nc.gpsimd.dma_start(out=Br[32 * g:32 * g + 3, :], in_=Pe[:, s, :])
                nc.gpsimd.dma_start(out=Cr[32 * g:32 * g + 3, :], in_=Pe[:, s, :])
                nc.gpsimd.dma_start(out=Cr[32 * g + 3:32 * g + 6, :], in_=Pe2s[:, :])
            advance(hooks)

        pending = None
        for s in range(BS):
            mlp(s, pending)
            if pending is not None:
                for _ in pending:
                    pass
            if do_gram:
                pending = gram_rounds(s)
        if pending is not None:
            for _ in pending:
                pass

        # ---------- finale ----------
        # cross term sum(gt*est) per coordinate -> Ft col 4 (in-place dump into Pe)
        nc.vector.scalar_tensor_tensor(out=Pe[:, :, :], in0=Pg[:, :, :],
                                       scalar=0.0, in1=Pe[:, :, :],
                                       op0=OP.add, op1=OP.mult,
                                       accum_out=Ft[0:3, 4:5])
        nc.vector.tensor_reduce(out=Ft[0:3, 2:3], in_=SGS[:, :], axis=AX.X, op=OP.add)
        nc.vector.tensor_reduce(out=Ft[0:3, 3:4], in_=SES[:, :], axis=AX.X, op=OP.add)
        nc.vector.tensor_reduce(out=Ft[:, 0:1], in_=M1[:, :], axis=AX.X, op=OP.add)
        nc.vector.tensor_reduce(out=Ft[:, 1:2], in_=M2[:, :], axis=AX.X, op=OP.add)

        fps = psG.tile([128, 1024], F32, tag="g", name="fps")
        nc.tensor.matmul(fps[0:1, 0:8], ones_c[:, :], Ft[:, :],
                         start=True, stop=True)
        nc.scalar.activation(outs[:, :], fps[0:1, 0:8], ACTF.Copy)
        nc.sync.dma_start(out=out_d, in_=outs)

    nc.compile()
    return nc


_program_cache = []


def kernel(**inputs):
    global LAST
    if not _program_cache:
        _program_cache.append(build_program())
    nc = _program_cache[0]

    def f32(x):
        return np.ascontiguousarray(np.asarray(x, dtype=np.float32))

    W1 = np.asarray(inputs["W1"], np.float32)
    W2 = np.asarray(inputs["W2"], np.float32)
    W3 = np.asarray(inputs["W3"], np.float32)
    W4 = np.asarray(inputs["W4"], np.float32)
    FP8 = ml_dtypes.float8_e4m3fn

    # L1 DoubleRow operands: slots (p, k): (0,0)=x (1,0)=y (0,1)=z (1,1)=0
    w1a = np.zeros((2, 2, 512), dtype=FP8)
    w1a[0, 0] = W1[0].astype(FP8)
    w1a[1, 0] = W1[1].astype(FP8)
    w1a[0, 1] = W1[2].astype(FP8)

    shared = {
        "w1a": w1a,
        "eye3": np.eye(3, dtype=ml_dtypes.bfloat16),
        "w1l": np.ascontiguousarray(
            W1[3:259, :].reshape(2, 128, 512).transpose(1, 0, 2).astype(ml_dtypes.bfloat16)),
        "b1r": f32(np.asarray(inputs["b1"], np.float32).reshape(1, 512)),
        "w2p": np.ascontiguousarray(W2.reshape(4, 128, 512).transpose(1, 0, 2).astype(FP8)),
        "b2p": f32(np.asarray(inputs["b2"], np.float32).reshape(4, 128).T),
        "w3p": np.ascontiguousarray(W3.reshape(4, 128, 256).transpose(1, 0, 2).astype(FP8)),
        "b3p": f32(np.asarray(inputs["b3"], np.float32).reshape(2, 128).T),
        "w4p": np.concatenate([W4.reshape(2, 128, 3).transpose(1, 0, 2).astype(FP8), np.zeros((128, 2, 13), dtype=FP8)], axis=2),
        "b4p": f32(np.asarray(inputs["b4"], np.float32).reshape(3, 1)),
    }
    in_maps = []
    for c in range(NCORES):
        sl = slice(c * BS, (c + 1) * BS)
        m = dict(shared)
        obs_c = np.asarray(inputs["obs"][sl], np.float32).transpose(2, 0, 1)  # [3,BS,N]
        obs_b = obs_c.astype(ml_dtypes.bfloat16)
        m["obs_t"] = np.ascontiguousarray(obs_b)
        obs8 = np.zeros((2, 2, BS, N), dtype=FP8)
        obs8[0, 0] = obs_b[0].astype(FP8)
        obs8[1, 0] = obs_b[1].astype(FP8)
        obs8[0, 1] = obs_b[2].astype(FP8)
        m["obs8"] = obs8
        m["gt_t"] = np.ascontiguousarray(np.asarray(inputs["obs_gt"][sl], np.float32).transpose(2, 0, 1).astype(ml_dtypes.bfloat16))
        m["lat_t"] = np.ascontiguousarray(np.asarray(inputs["latent"][sl], np.float32).T.astype(ml_dtypes.bfloat16))
        in_maps.append(m)

    res = run_bass_kernel_spmd(nc, in_maps, core_ids=list(range(NCORES)),
                               trace=TRACE)
    LAST = res

    parts = np.stack([r["partials"][0] for r in res.results]).astype(np.float64)
    s_max1 = parts[:, 0].sum()
    s_max2 = parts[:, 1].sum()
    s_gt2 = parts[:, 2].sum()
    s_est2 = parts[:, 3].sum()
    s_cross = parts[:, 4].sum()
    chm = (s_gt2 - 2.0 * s_max1) / (B * N) + (s_est2 - 2.0 * s_max2) / (B * N)
    l2 = (s_gt2 - 2.0 * s_cross + s_est2) / (B * N * 3)
    loss = 0.2 * chm + 0.8 * l2
    return np.asarray(loss, dtype=np.float32)



# revision 17
# speedup vs baseline: 1.4030x; 1.0754x over previous
"""Trainium2 Bass kernel for nn_DeepLatent loss (chamfer + L2 of a per-point MLP).

Strategy (8 cores, data-parallel over batch B=32 -> 4 samples/core):
  MLP in fp8e4 DoubleRow (2x contraction/instr): activations + weights e4m3,
  fp32 PSUM accumulation, biases folded into the ACT pass.
    h1 = relu(W1a.T @ obs8 + latbias)   latbias = W1lat.T @ latent + b1
    h2 = relu(W2.T @ h1), h3 = relu(W3.T @ h2), est = W4.T @ h3 + b4 + obs

  Chamfer via ONE augmented gram per sample producing psum = -0.5*d2[n,m]:
    lhsT rows (host-built, 32s-strided): [gt_x, gt_y, gt_z, -0.5*|gt|^2, -.5,-.5,-.5]
    rhs  rows (device-built):            [est_x, est_y, est_z, 1, ex^2, ey^2, ez^2]
    n-dir min: M1[n] = max_m psum  (DVE tensor_tensor_reduce over j-half pairs)
    m-dir min: M2[m] = max_n psum  (Pool/DVE running col-max + partition_all_reduce)
    min d2 = -2 * max(-0.5 d2); scalar partials combined on the host.
"""

import ml_dtypes
import numpy as np
from contextlib import ExitStack

import concourse.bass as bass
import concourse.bacc as bacc
import concourse.mybir as mybir
import concourse.tile as tile
from concourse.bass_isa import ReduceOp
from concourse.bass_utils import run_bass_kernel_spmd

F32 = mybir.dt.float32
BF16 = mybir.dt.bfloat16
F8 = mybir.dt.float8e4
DR = mybir.MatmulPerfMode.DoubleRow
AX = mybir.AxisListType
OP = mybir.AluOpType
ACTF = mybir.ActivationFunctionType

B, N, L = 32, 1024, 256
NCORES = 8
BS = B // NCORES  # samples per core
NT = N // 128     # n-tiles per sample
NEG = -3.0e38

# test.py hooks
TRACE = False
LAST = None


def build_program():
    nc = bacc.Bacc()

    obs8_d = nc.dram_tensor("obs8", [2, 2, BS, N], F8, kind="ExternalInput")[:]
    obsp_d = nc.dram_tensor("obsp", [128, N], BF16, kind="ExternalInput")[:]
    gta_d = nc.dram_tensor("gta", [128, N], BF16, kind="ExternalInput")[:]
    one_d = nc.dram_tensor("onesrow", [1, N], BF16, kind="ExternalInput")[:]
    lat_d = nc.dram_tensor("lat_t", [L, BS], BF16, kind="ExternalInput")[:]
    W1ad = nc.dram_tensor("w1a", [2, 2, 512], F8, kind="ExternalInput")[:]
    W1ld = nc.dram_tensor("w1l", [128, 2, 512], BF16, kind="ExternalInput")[:]
    b1d = nc.dram_tensor("b1r", [1, 512], F32, kind="ExternalInput")[:]
    W2d = nc.dram_tensor("w2p", [128, 4, 512], F8, kind="ExternalInput")[:]
    b2d = nc.dram_tensor("b2p", [128, 4], F32, kind="ExternalInput")[:]
    W3d = nc.dram_tensor("w3p", [128, 4, 256], F8, kind="ExternalInput")[:]
    b3d = nc.dram_tensor("b3p", [128, 2], F32, kind="ExternalInput")[:]
    W4d = nc.dram_tensor("w4p", [128, 2, 16], F8, kind="ExternalInput")[:]
    b4d = nc.dram_tensor("b4p", [3, 1], F32, kind="ExternalInput")[:]
    out_d = nc.dram_tensor("partials", [1, 12], F32, kind="ExternalOutput")[:]

    with tile.TileContext(nc) as tc, ExitStack() as ctx:
        singles = ctx.enter_context(tc.tile_pool(name="singles", bufs=1))

        def fixed(shape, name, dtype=F32):
            return singles.tile(shape, dtype, tag=name, name=name)

        # ---------- fixed tiles ----------
        w1a = fixed([2, 2, 512], "w1a", F8)
        obs8 = fixed([2, 2, BS, N], "obs8", F8)
        w1l = fixed([128, 2, 512], "w1l", BF16)
        b1r = fixed([1, 512], "b1r")
        w2t = fixed([128, 4, 512], "w2t", F8)
        w3t = fixed([128, 4, 256], "w3t", F8)
        w4t = fixed([128, 2, 16], "w4t", F8)
        b2t = fixed([128, 4], "b2t")
        b3t = fixed([128, 2], "b3t")
        b4p = fixed([3, 1], "b4p")
        latT = fixed([128, 2, BS], "latT", BF16)
        ones_r = fixed([1, BS], "ones_r")
        ones_c = fixed([128, 1], "ones_c")
        latb = fixed([128, 4, BS], "latb")
        gtaS = [fixed([7, N], f"gta{i}", BF16) for i in range(BS)]
        obspS = [fixed([3, N], f"obsp{i}", BF16) for i in range(BS)]
        estbS = [fixed([7, N], f"estb{i}", BF16) for i in range(BS)]
        sqtS = [fixed([3, N], f"sqt{i}", BF16) for i in range(BS)]
        Pe = fixed([3, BS, N], "Pe", BF16)
        RM = [fixed([128, 1024], f"RM{i}") for i in range(2)]
        PARD = [fixed([128, 1024], f"PARD{i}") for i in range(2)]
        M1 = fixed([128, 32], "M1")
        M2c = fixed([1, 4], "M2c")
        Ft = fixed([128, 12], "Ft")
        ttrd = fixed([128, 512], "ttrd", BF16)
        outs = fixed([1, 12], "outs")

        h1p = ctx.enter_context(tc.tile_pool(name="h1", bufs=2))
        h2p = ctx.enter_context(tc.tile_pool(name="h2", bufs=2))
        h3p = ctx.enter_context(tc.tile_pool(name="h3", bufs=2))
        psA = ctx.enter_context(tc.tile_pool(name="psA", bufs=2, space="PSUM"))
        psG = ctx.enter_context(tc.tile_pool(name="psG", bufs=2, space="PSUM"))

        # ---------- startup ----------
        nc.sync.dma_start(out=w1a, in_=W1ad)
        nc.sync.dma_start(out=obs8, in_=obs8_d)
        for i in range(BS):
            nc.sync.dma_start(out=gtaS[i], in_=gta_d[32 * i:32 * i + 7, :])
            nc.sync.dma_start(out=obspS[i], in_=obsp_d[32 * i:32 * i + 3, :])
        for k in range(2):
            nc.sync.dma_start(out=latT[:, k, :], in_=lat_d[128 * k:128 * (k + 1), :])
        nc.sync.dma_start(out=b1r, in_=b1d)
        nc.sync.dma_start(out=w1l, in_=W1ld)
        nc.sync.dma_start(out=b2t, in_=b2d)
        nc.sync.dma_start(out=b3t, in_=b3d)
        nc.sync.dma_start(out=b4p, in_=b4d)
        nc.sync.dma_start(out=w2t, in_=W2d)
        nc.sync.dma_start(out=w3t, in_=W3d)
        nc.sync.dma_start(out=w4t, in_=W4d)
        nc.vector.memset(ones_r, 1.0)
        nc.vector.memset(ones_c, 1.0)
        nc.vector.memset(Ft, 0.0)
        # estb "ones" aux row per sample (pairs with the -0.5*|gt|^2 lhsT row)
        for i in range(BS):
            nc.sync.dma_start(out=estbS[i][3:4, :], in_=one_d)

        # latent bias vectors: latb[cout, c-tile, s] = (latent @ W1[3:] + b1)^T
        for c in range(4):
            lps = psA.tile([128, N], F32, tag="a", name=f"latps{c}")
            for k in range(2):
                nc.tensor.matmul(lps[:, 0:BS], w1l[:, k, 128 * c:128 * (c + 1)],
                                 latT[:, k, :], start=(k == 0), stop=False)
            nc.tensor.matmul(lps[:, 0:BS], b1r[:, 128 * c:128 * (c + 1)],
                             ones_r[:, :], start=False, stop=True)
            nc.vector.tensor_copy(latb[:, c, :], lps[:, 0:BS])

        # ---------- per-sample gram rounds (generator; interleaved with next MLP) ----------
        def gram_rounds(s):
            rm = RM[s % 2]
            pard = PARD[s % 2]
            lhs = gtaS[s]
            rhs = estbS[s]
            for t in range(NT):
                gp = psG.tile([128, 1024], F32, tag="g", name=f"gp{s}_{t}")
                for j in range(2):
                    nc.tensor.matmul(gp[:, 512 * j:512 * (j + 1)],
                                     lhs[:, 128 * t:128 * (t + 1)],
                                     rhs[:, 512 * j:512 * (j + 1)],
                                     start=True, stop=True)
                # n-dir: M1[n, 8s+t] = max_m (whole tile) on DVE
                nc.vector.tensor_reduce(out=M1[:, 8 * s + t:8 * s + t + 1],
                                        in_=gp[:, :], axis=AX.X, op=OP.max)
                # m-dir: running column max on DVE (Pool cannot read PSUM)
                if t == 0:
                    nc.vector.tensor_copy(rm[:, :], gp[:, :])
                else:
                    nc.vector.scalar_tensor_tensor(
                        out=rm[:, :], in0=gp[:, :], scalar=0.0,
                        in1=rm[:, :], op0=OP.add, op1=OP.max)
                yield
            nc.gpsimd.partition_all_reduce(pard, rm, 128, ReduceOp.max)
            yield
            nc.vector.tensor_reduce(out=M2c[0:1, s:s + 1], in_=pard[0:1, :],
                                    axis=AX.X, op=OP.add)
            yield

        def advance(it):
            if it is not None:
                next(it, None)

        # ---------- per-sample MLP ----------
        def mlp(s, hooks):
            h1t = h1p.tile([128, 4, N], F8, tag="h1", name=f"h1_{s}")
            for c in range(4):
                ps = psA.tile([128, N], F32, tag="a", name=f"l1ps{s}_{c}")
                for j in range(2):
                    nc.tensor.matmul(ps[:, 512 * j:512 * (j + 1)],
                                     w1a[:, :, 128 * c:128 * (c + 1)],
                                     obs8[:, :, s, 512 * j:512 * (j + 1)],
                                     start=True, stop=True, perf_mode=DR)
                nc.scalar.activation(h1t[:, c, :], ps[:, :], ACTF.Relu,
                                     bias=latb[:, c, s:s + 1])
                advance(hooks)

            h2t = h2p.tile([128, 4, N], F8, tag="h2", name=f"h2_{s}")
            for c in range(4):
                ps = psA.tile([128, N], F32, tag="a", name=f"l2ps{s}_{c}")
                for j in range(2):
                    for k in range(2):
                        nc.tensor.matmul(ps[:, 512 * j:512 * (j + 1)],
                                         w2t[:, 2 * k:2 * k + 2, 128 * c:128 * (c + 1)],
                                         h1t[:, 2 * k:2 * k + 2, 512 * j:512 * (j + 1)],
                                         start=(k == 0), stop=(k == 1), perf_mode=DR)
                nc.scalar.activation(h2t[:, c, :], ps[:, :], ACTF.Relu,
                                     bias=b2t[:, c:c + 1])
                advance(hooks)

            h3t = h3p.tile([128, 2, N], F8, tag="h3", name=f"h3_{s}")
            for c in range(2):
                ps = psA.tile([128, N], F32, tag="a", name=f"l3ps{s}_{c}")
                for j in range(2):
                    for k in range(2):
                        nc.tensor.matmul(ps[:, 512 * j:512 * (j + 1)],
                                         w3t[:, 2 * k:2 * k + 2, 128 * c:128 * (c + 1)],
                                         h2t[:, 2 * k:2 * k + 2, 512 * j:512 * (j + 1)],
                                         start=(k == 0), stop=(k == 1), perf_mode=DR)
                nc.scalar.activation(h3t[:, c, :], ps[:, :], ACTF.Relu,
                                     bias=b3t[:, c:c + 1])
                advance(hooks)

            ps4 = psA.tile([128, N], F32, tag="a", name=f"l4ps{s}")
            for j in range(2):
                nc.tensor.matmul(ps4[0:3, 512 * j:512 * (j + 1)],
                                 w4t[:, :, 0:3],
                                 h3t[:, :, 512 * j:512 * (j + 1)],
                                 start=True, stop=True, perf_mode=DR)
            nc.scalar.activation(Pe[:, s, :], ps4[0:3, :], ACTF.Identity,
                                 bias=b4p[:, 0:1])
            advance(hooks)
            # est rows into the gram rhs block, then + obs (bf16), squares, aux
            eb = estbS[s][0:3, :]
            nc.sync.dma_start(out=eb, in_=Pe[:, s, :])
            nc.vector.scalar_tensor_tensor(out=eb, in0=eb, scalar=0.0,
                                           in1=obspS[s], op0=OP.add, op1=OP.add)
            nc.scalar.activation(sqtS[s], eb, ACTF.Square,
                                 accum_out=Ft[0:3, 4 + s:5 + s])
            nc.sync.dma_start(out=estbS[s][4:7, :], in_=sqtS[s])
            advance(hooks)
            # cross term sum(gt * est) for the L2 part
            nc.vector.scalar_tensor_tensor(out=sqtS[s],
                                           in0=gtaS[s][0:3, :],
                                           scalar=0.0, in1=eb,
                                           op0=OP.add, op1=OP.mult,
                                           accum_out=Ft[0:3, 8 + s:9 + s])
            advance(hooks)

        pending = None
        for s in range(BS):
            mlp(s, pending)
            if pending is not None:
                for _ in pending:
                    pass
            pending = gram_rounds(s)
        if pending is not None:
            for _ in pending:
                pass

        # ---------- finale ----------
        nc.vector.tensor_reduce(out=Ft[:, 0:1], in_=M1[:, :], axis=AX.X, op=OP.add)
        nc.vector.tensor_reduce(out=Ft[0:1, 1:2], in_=M2c[:, :], axis=AX.X, op=OP.add)

        fps = psA.tile([128, N], F32, tag="a", name="fps")
        nc.tensor.matmul(fps[0:1, 0:12], ones_c[:, :], Ft[:, :],
                         start=True, stop=True)
        nc.scalar.activation(outs[:, :], fps[0:1, 0:12], ACTF.Copy)
        nc.sync.dma_start(out=out_d, in_=outs)

    nc.compile()
    return nc


_program_cache = []


def kernel(**inputs):
    global LAST
    if not _program_cache:
        _program_cache.append(build_program())
    nc = _program_cache[0]

    def f32(x):
        return np.ascontiguousarray(np.asarray(x, dtype=np.float32))

    W1 = np.asarray(inputs["W1"], np.float32)
    W2 = np.asarray(inputs["W2"], np.float32)
    W3 = np.asarray(inputs["W3"], np.float32)
    W4 = np.asarray(inputs["W4"], np.float32)
    FP8 = ml_dtypes.float8_e4m3fn
    BF = ml_dtypes.bfloat16

    # L1 DoubleRow operands: slots (p, k): (0,0)=x (1,0)=y (0,1)=z (1,1)=0
    w1a = np.zeros((2, 2, 512), dtype=FP8)
    w1a[0, 0] = W1[0].astype(FP8)
    w1a[1, 0] = W1[1].astype(FP8)
    w1a[0, 1] = W1[2].astype(FP8)

    w4p = np.zeros((128, 2, 16), dtype=FP8)
    w4p[:, :, 0:3] = W4.reshape(2, 128, 3).transpose(1, 0, 2).astype(FP8)

    shared = {
        "w1a": w1a,
        "w1l": np.ascontiguousarray(
            W1[3:259, :].reshape(2, 128, 512).transpose(1, 0, 2).astype(BF)),
        "b1r": f32(np.asarray(inputs["b1"], np.float32).reshape(1, 512)),
        "w2p": np.ascontiguousarray(W2.reshape(4, 128, 512).transpose(1, 0, 2).astype(FP8)),
        "b2p": f32(np.asarray(inputs["b2"], np.float32).reshape(4, 128).T),
        "w3p": np.ascontiguousarray(W3.reshape(4, 128, 256).transpose(1, 0, 2).astype(FP8)),
        "b3p": f32(np.asarray(inputs["b3"], np.float32).reshape(2, 128).T),
        "w4p": w4p,
        "b4p": f32(np.asarray(inputs["b4"], np.float32).reshape(3, 1)),
        "onesrow": np.ones((1, N), dtype=BF),
    }
    in_maps = []
    s_gt2 = np.zeros(NCORES, dtype=np.float64)
    for c in range(NCORES):
        sl = slice(c * BS, (c + 1) * BS)
        m = dict(shared)
        obs_c = np.asarray(inputs["obs"][sl], np.float32).transpose(2, 0, 1)  # [3,BS,N]
        obs_b = obs_c.astype(BF)
        obs8 = np.zeros((2, 2, BS, N), dtype=FP8)
        obs8[0, 0] = obs_b[0].astype(FP8)
        obs8[1, 0] = obs_b[1].astype(FP8)
        obs8[0, 1] = obs_b[2].astype(FP8)
        m["obs8"] = obs8
        obsp = np.zeros((128, N), dtype=BF)
        gt_b = np.asarray(inputs["obs_gt"][sl], np.float32).transpose(2, 0, 1).astype(BF)
        gta = np.zeros((128, N), dtype=BF)
        for s in range(BS):
            obsp[32 * s:32 * s + 3] = obs_b[:, s, :]
            gta[32 * s:32 * s + 3] = gt_b[:, s, :]
            gta[32 * s + 3] = (-0.5 * np.square(gt_b[:, s, :].astype(np.float32)).sum(0)).astype(BF)
            gta[32 * s + 4:32 * s + 7] = np.asarray(-0.5, dtype=BF)
        m["obsp"] = obsp
        m["gta"] = gta
        s_gt2[c] = np.square(gt_b.astype(np.float64)).sum()
        m["lat_t"] = np.ascontiguousarray(
            np.asarray(inputs["latent"][sl], np.float32).T.astype(BF))
        in_maps.append(m)

    res = run_bass_kernel_spmd(nc, in_maps, core_ids=list(range(NCORES)),
                               trace=TRACE)
    LAST = res

    parts = np.stack([r["partials"][0] for r in res.results]).astype(np.float64)
    s_max1 = parts[:, 0].sum()
    s_max2 = parts[:, 1].sum()
    s_est2 = parts[:, 4:8].sum()
    s_cross = parts[:, 8:12].sum()
    gt2 = s_gt2.sum()
    chm = (-2.0 * s_max1 - 2.0 * s_max2) / (B * N)
    l2 = (gt2 - 2.0 * s_cross + s_est2) / (B * N * 3)
    loss = 0.2 * chm + 0.8 * l2
    return np.asarray(loss, dtype=np.float32)
